# revision 19
# baseline (speedup 1.0000x reference)
"""LSTM-pool kernel for Trainium2, 8-core data-parallel SPMD.

Math (per batch row b):
  x_t = [seq[b,t], seq_e[b,t], seq_t[b,t]]              (A = 384)
  z_t = x_t @ Wi + h_{t-1} @ Wh + bh                    (4F = 512, gates i,f,g,o)
  c_t = sig(f)*c_{t-1} + sig(i)*tanh(g);  h_t = sig(o)*tanh(c_t)
  out = relu([h_T, src] @ W1 + b1) @ W2 + b2

Design notes:
- Everything on-device runs transposed: feature/gate dim on partitions, batch
  on the free dim.  The host pre-transposes each core's inputs to [F, T, B]
  and pre-casts them (x and Wi to fp8 e4m3, the rest to bf16), so the kernel
  issues plain HWDGE loads with zero on-chip transposes or casts.
- Gate columns are host-permuted to [i, f, o, g] and the g columns of Wi/Wh
  (and bh) are pre-scaled by 2 so that one sigmoid instruction covers all
  four gate quadrants: tanh(g) = 2*sigmoid(2g) - 1, fixed up by a single
  fused DVE tensor_scalar (x*2-1).
- The input projection x @ Wi uses fp8 DoubleRow matmuls (K=256 packed over
  two of the three 128-chunks of A, plus one plain fp8 K=128 matmul).
- Batch 512 per core is processed as two interleaved halves of 256 whose
  recurrent chains are software-pipelined half a step apart (half B's
  tanh(c)/h for step t-1 are emitted inside iteration t) so the strict-FIFO
  ACT/DVE queues never serialize the two chains.
"""

import sys

sys.path.insert(0, "/opt/trn_rl_repo")

import numpy as np
import ml_dtypes

import concourse.bass as bass
import concourse.mybir as mybir
import concourse.tile as tile
from concourse import bacc
from concourse.bass_utils import run_bass_kernel_spmd

dt = mybir.dt
AF = mybir.ActivationFunctionType
ALU = mybir.AluOpType
PM = mybir.MatmulPerfMode

NCORES = 8
BFULL = 4096
B = BFULL // NCORES  # 512 batch rows per core
T = 128
F = 128
G = 512  # 4F
TC = 16  # time steps per DMA chunk
NH = B // 2  # half-batch = 256

USE_FP8 = True  # x + Wi in fp8e4m3 with DoubleRow matmuls; else bf16
DEBUG_DUMP = False  # add h/c state dram outputs (CoreSim debugging)

E4 = ml_dtypes.float8_e4m3
BF = ml_dtypes.bfloat16


def build_nc(zero_bias: bool):
    nc = bacc.Bacc("TRN2", target_bir_lowering=False, debug=False, num_devices=NCORES)

    xdt = dt.float8e4 if USE_FP8 else dt.bfloat16
    seq = nc.dram_tensor("seq", [F, T, B], xdt, kind="ExternalInput")
    seq_e = nc.dram_tensor("seq_e", [F, T, B], xdt, kind="ExternalInput")
    seq_t = nc.dram_tensor("seq_t", [F, T, B], xdt, kind="ExternalInput")
    srcT = nc.dram_tensor("srcT", [F, B], dt.bfloat16, kind="ExternalInput")
    Wi = nc.dram_tensor("Wi", [128, 3, G], xdt, kind="ExternalInput")
    Wh = nc.dram_tensor("Wh", [F, G], dt.bfloat16, kind="ExternalInput")
    bh = nc.dram_tensor("bh", [128, 4], dt.float32, kind="ExternalInput")
    W1 = nc.dram_tensor("W1", [128, 2, F], dt.bfloat16, kind="ExternalInput")
    b1 = nc.dram_tensor("b1", [F, 1], dt.float32, kind="ExternalInput")
    W2 = nc.dram_tensor("W2", [F, F], dt.bfloat16, kind="ExternalInput")
    b2 = nc.dram_tensor("b2", [F, 1], dt.float32, kind="ExternalInput")
    outT = nc.dram_tensor("outT", [F, B], dt.float32, kind="ExternalOutput")

    xdram = [seq, seq_e, seq_t]
    nchunk = T // TC

    with tile.TileContext(nc) as tc:
        with (
            tc.tile_pool(name="const", bufs=1) as constp,
            tc.tile_pool(name="xt", bufs=2) as xtp,
            tc.tile_pool(name="gates", bufs=2) as gatep,
        ):
            # ---------------- critical-path loads first ----------------
            # Wi, then the first few timesteps of x, then Wh — so the first
            # input-projection matmuls start ~12us earlier than if all the
            # small constant loads queued ahead of the x data.
            wi = constp.tile([128, 3, G], xdt)
            nc.sync.dma_start(wi[:], Wi[:])

            xtiles = {}

            def load_chunk(ch, split=1):
                xt = xtp.tile([128, 3, TC, B], xdt, tag="xt", name=f"xt_{ch}")
                sub = TC // split
                for s in range(split):
                    tsl = slice(s * sub, (s + 1) * sub)
                    for kc, dram in enumerate(xdram):
                        # prologue loads alternate between the two HWDGE
                        # rings (SP + ACT) so they drain in parallel
                        eng = nc.scalar if (split > 1 and (s * 3 + kc) % 2) else nc.sync
                        eng.dma_start(
                            xt[:, kc, tsl, :],
                            dram[:, ch * TC + s * sub : ch * TC + (s + 1) * sub, :],
                        )
                xtiles[ch] = xt

            load_chunk(0, split=8)

            wh = constp.tile([128, G], dt.bfloat16)
            nc.sync.dma_start(wh[:], Wh[:])

            # ---------------- remaining constants ----------------
            w1 = constp.tile([128, 2, F], dt.bfloat16)
            nc.sync.dma_start(w1[:], W1[:])
            w2 = constp.tile([128, F], dt.bfloat16)
            nc.sync.dma_start(w2[:], W2[:])
            b1t = constp.tile([128, 1], dt.float32)
            nc.sync.dma_start(b1t[:], b1[:])
            b2t = constp.tile([128, 1], dt.float32)
            nc.sync.dma_start(b2t[:], b2[:])
            bh4 = constp.tile([128, 4], dt.float32)
            nc.sync.dma_start(bh4[:], bh[:])
            srt = constp.tile([128, B], dt.bfloat16)
            nc.sync.dma_start(srt[:], srcT[:])

            # ---------------- persistent state ----------------
            cs = []
            hs = []
            for hf in range(2):
                c_h = constp.tile([128, NH], dt.bfloat16, name=f"c_{hf}")
                nc.gpsimd.memset(c_h[:], 0.0)
                cs.append(c_h)
                h_h = constp.tile([128, NH], dt.bfloat16, name=f"h_{hf}")
                nc.gpsimd.memset(h_h[:], 0.0)
                hs.append(h_h)

            def wi_mms(z, xt, ts, hf):
                """Input projection for one half-step into a [128,4,NH] z tile.

                Banks: quads (0,1) share bank 0, quads (2,3) share bank 1 —
                start=True only on the first matmul touching each bank; stop
                comes later from the half's own Wh matmuls.
                """
                hsl = slice(hf * NH, (hf + 1) * NH)
                for q in range(4):
                    qs = slice(q * 128, (q + 1) * 128)
                    first = q in (0, 2)
                    if USE_FP8:
                        nc.tensor.matmul(
                            z[:, q, :],
                            wi[:, 0:2, qs],
                            xt[:, 0:2, ts, hsl],
                            start=first,
                            stop=False,
                            perf_mode=PM.DoubleRow,
                        )
                        nc.tensor.matmul(
                            z[:, q, :],
                            wi[:, 2, qs],
                            xt[:, 2, ts, hsl],
                            start=False,
                            stop=False,
                        )
                    else:
                        for kc in range(3):
                            nc.tensor.matmul(
                                z[:, q, :],
                                wi[:, kc, qs],
                                xt[:, kc, ts, hsl],
                                start=(first and kc == 0),
                                stop=False,
                            )

            def wh_mms(z, hf):
                for q in range(4):
                    nc.tensor.matmul(
                        z[:, q, :],
                        wh[:, q * 128 : (q + 1) * 128],
                        hs[hf][:],
                        start=False,
                        stop=(q in (1, 3)),  # last matmul of each bank
                    )

            def sig_gates(z, hf, t):
                """sigmoid over all 4 quadrants of one half's z -> sg tile."""
                sg = gatep.tile(
                    [128, 4, NH], dt.bfloat16, tag=f"sg{hf}", name=f"sg{hf}_{t}"
                )
                if zero_bias:
                    nc.scalar.activation(sg[:], z[:], AF.Sigmoid)
                else:
                    for q in range(4):
                        nc.scalar.activation(
                            sg[:, q, :],
                            z[:, q, :],
                            AF.Sigmoid,
                            bias=bh4[:, q : q + 1],
                        )
                return sg

            def cell_update(sg, hf, t):
                """tg = 2*sig(2g)-1; c = sig(f)*c + sig(i)*tg  (DVE only)."""
                tg = gatep.tile([128, NH], dt.bfloat16, tag=f"tg{hf}", name=f"tg{hf}_{t}")
                nc.vector.tensor_scalar(tg[:], sg[:, 3, :], 2.0, -1.0, ALU.mult, ALU.add)
                m1 = gatep.tile([128, NH], dt.bfloat16, tag=f"m1{hf}", name=f"m1{hf}_{t}")
                nc.vector.tensor_mul(m1[:], sg[:, 1, :], cs[hf][:])
                m2 = gatep.tile([128, NH], dt.bfloat16, tag=f"m2{hf}", name=f"m2{hf}_{t}")
                nc.vector.tensor_mul(m2[:], sg[:, 0, :], tg[:])
                nc.vector.tensor_add(cs[hf][:], m1[:], m2[:])

            def h_update(sg, hf, t):
                """tc = tanh(c) (ACT); h = sig(o)*tc (DVE, split in halves)."""
                tc_ = gatep.tile([128, NH], dt.bfloat16, tag=f"tc{hf}", name=f"tc{hf}_{t}")
                nc.scalar.activation(tc_[:], cs[hf][:], AF.Tanh)
                nc.vector.tensor_mul(hs[hf][:], sg[:, 2, :], tc_[:])

            # ---------------- main loop ----------------
            zp_ctx = tc.tile_pool(name="zp", bufs=2, space="PSUM")
            zp = zp_ctx.__enter__()

            def z_tile(hf, t):
                return zp.tile(
                    [128, 4, NH], dt.float32, tag=f"z{hf}", name=f"z{hf}_{t}"
                )

            # HAM warm-up: keep the PE busy with throwaway matmuls on the
            # zero h/c tiles while the first x chunk streams in, so the
            # prologue input projections run at 2.4 GHz instead of 1.2.
            warm = zp.tile([128, 4, NH], dt.float32, tag="z0", name="warm")
            NWARM = 15
            for r in range(NWARM):
                for q in range(4):
                    nc.tensor.matmul(
                        warm[:, q, :],
                        hs[0][:, 0:128],
                        hs[1][:],
                        start=(r == 0 and q in (0, 2)),
                        stop=(r == NWARM - 1 and q in (1, 3)),
                    )

            zA_cur = z_tile(0, 0)
            wi_mms(zA_cur, xtiles[0], 0, 0)
            zB_cur = z_tile(1, 0)
            wi_mms(zB_cur, xtiles[0], 0, 1)

            sgB_prev = None
            for t in range(T):
                ch, ts = divmod(t, TC)
                if ts == 0 and ch + 1 < nchunk:
                    load_chunk(ch + 1)

                # half A, step t: recurrent matmul + gates + cell update
                wh_mms(zA_cur, 0)
                sgA = sig_gates(zA_cur, 0, t)
                cell_update(sgA, 0, t)

                # input projection for half A, step t+1 (fills PE idle slot)
                if t + 1 < T:
                    zA_next = z_tile(0, t + 1)
                    wi_mms(zA_next, xtiles[(t + 1) // TC], (t + 1) % TC, 0)

                # half B, step t-1: tanh(c)/h displaced into this iteration
                if sgB_prev is not None:
                    h_update(sgB_prev, 1, t - 1)

                # half B, step t: recurrent matmul
                wh_mms(zB_cur, 1)

                # half A, step t: h update
                h_update(sgA, 0, t)

                sgB = sig_gates(zB_cur, 1, t)
                cell_update(sgB, 1, t)
                sgB_prev = sgB

                # input projection for half B, step t+1
                if t + 1 < T:
                    zB_next = z_tile(1, t + 1)
                    wi_mms(zB_next, xtiles[(t + 1) // TC], (t + 1) % TC, 1)
                    zA_cur = zA_next
                    zB_cur = zB_next

            # epilogue: half B's final h
            h_update(sgB_prev, 1, T - 1)

            zp_ctx.__exit__(None, None, None)

            if DEBUG_DUMP:
                for nm, tiles in (("h", hs), ("c", cs)):
                    dbg = nc.dram_tensor(
                        f"dbg_{nm}", [F, B], dt.float32, kind="ExternalOutput"
                    )
                    sb = constp.tile([128, B], dt.float32, name=f"dbg_{nm}_sb")
                    for hf in range(2):
                        nc.vector.tensor_copy(
                            sb[:, hf * NH : (hf + 1) * NH], tiles[hf][:]
                        )
                    nc.sync.dma_start(dbg[:], sb[:])

            # ---------------- merge layer ----------------
            with tc.tile_pool(name="mp", bufs=1, space="PSUM") as mp:
                ps_hid = mp.tile([128, B], dt.float32)
                for hf in range(2):
                    nc.tensor.matmul(
                        ps_hid[:, hf * NH : (hf + 1) * NH],
                        w1[:, 0, :],
                        hs[hf][:],
                        start=(hf == 0),
                        stop=False,
                    )
                nc.tensor.matmul(ps_hid[:], w1[:, 1, :], srt[:], start=False, stop=True)
                hid_bf = constp.tile([128, B], dt.bfloat16)
                nc.scalar.activation(hid_bf[:], ps_hid[:], AF.Relu, bias=b1t[:])

                ps_out = mp.tile([128, B], dt.float32)
                nc.tensor.matmul(ps_out[:], w2[:], hid_bf[:], start=True, stop=True)
                out_sb = constp.tile([128, B], dt.float32)
                nc.scalar.activation(out_sb[:], ps_out[:], AF.Identity, bias=b2t[:])
                nc.sync.dma_start(outT[:], out_sb[:])

    nc.compile()
    return nc


_NC_CACHE: dict = {}


def _get_nc(zero_bias: bool):
    if zero_bias not in _NC_CACHE:
        _NC_CACHE[zero_bias] = build_nc(zero_bias)
    return _NC_CACHE[zero_bias]


# gate-column permutation: reference order [i, f, g, o] -> device [i, f, o, g],
# with the g columns pre-scaled by 2 (tanh(g) = 2*sig(2g) - 1 on device).
def _permute_gates(w, scale_g=True):
    i, f, g, o = np.split(np.asarray(w, dtype=np.float32), 4, axis=-1)
    return np.concatenate([i, f, o, (2.0 * g) if scale_g else g], axis=-1)


def make_in_maps(**inputs):
    """Host-side packing: per-core [F,T,B] transposed inputs + cast weights."""
    xdt = E4 if USE_FP8 else BF
    f32 = lambda x: np.asarray(x, dtype=np.float32)

    Wi = _permute_gates(f32(inputs["Wi"]))  # [384, 512]
    Wi_dev = np.ascontiguousarray(
        Wi.reshape(3, 128, G).transpose(1, 0, 2)
    ).astype(xdt)
    Wh_dev = np.ascontiguousarray(_permute_gates(f32(inputs["Wh"]))).astype(BF)
    bh_dev = np.ascontiguousarray(
        _permute_gates(f32(inputs["bh"]).reshape(1, 4 * F)).reshape(4, F).T
    ).astype(np.float32)  # [128, 4]
    W1_dev = np.ascontiguousarray(
        f32(inputs["W1"]).reshape(2, 128, F).transpose(1, 0, 2)
    ).astype(BF)
    W2_dev = np.ascontiguousarray(f32(inputs["W2"])).astype(BF)
    b1_dev = f32(inputs["b1"]).reshape(F, 1)
    b2_dev = f32(inputs["b2"]).reshape(F, 1)

    seq8 = np.asarray(inputs["seq"], dtype=np.float32).astype(xdt)
    seqe8 = np.asarray(inputs["seq_e"], dtype=np.float32).astype(xdt)
    seqt8 = np.asarray(inputs["seq_t"], dtype=np.float32).astype(xdt)
    src = f32(inputs["src"])

    shared = {
        "Wi": Wi_dev, "Wh": Wh_dev, "bh": bh_dev,
        "W1": W1_dev, "b1": b1_dev, "W2": W2_dev, "b2": b2_dev,
    }
    in_maps = []
    for c in range(NCORES):
        sl = slice(c * B, (c + 1) * B)
        m = dict(shared)
        m["seq"] = np.ascontiguousarray(seq8[sl].transpose(2, 1, 0))
        m["seq_e"] = np.ascontiguousarray(seqe8[sl].transpose(2, 1, 0))
        m["seq_t"] = np.ascontiguousarray(seqt8[sl].transpose(2, 1, 0))
        m["srcT"] = np.ascontiguousarray(src[sl].T).astype(BF)
        in_maps.append(m)
    return in_maps


def kernel(**inputs) -> np.ndarray:
    zero_bias = not np.any(np.asarray(inputs["bh"]))
    nc = _get_nc(zero_bias)
    in_maps = make_in_maps(**inputs)
    res = run_bass_kernel_spmd(nc, in_maps, core_ids=list(range(NCORES)))
    out = np.empty((BFULL, F), np.float32)
    for c in range(NCORES):
        out[c * B : (c + 1) * B] = res.results[c]["outT"].T
    return out


# revision 20
# speedup vs baseline: 1.0012x; 1.0012x over previous
"""LSTM-pool kernel for Trainium2, 8-core data-parallel SPMD.

Math (per batch row b):
  x_t = [seq[b,t], seq_e[b,t], seq_t[b,t]]              (A = 384)
  z_t = x_t @ Wi + h_{t-1} @ Wh + bh                    (4F = 512, gates i,f,g,o)
  c_t = sig(f)*c_{t-1} + sig(i)*tanh(g);  h_t = sig(o)*tanh(c_t)
  out = relu([h_T, src] @ W1 + b1) @ W2 + b2

Design notes:
- Everything on-device runs transposed: feature/gate dim on partitions, batch
  on the free dim.  The host pre-transposes each core's inputs to [F, T, B]
  and pre-casts them (x and Wi to fp8 e4m3, the rest to bf16), so the kernel
  issues plain HWDGE loads with zero on-chip transposes or casts.
- Gate columns are host-permuted to [i, f, o, g] and the g columns of Wi/Wh
  (and bh) are pre-scaled by 2 so that one sigmoid instruction covers all
  four gate quadrants: tanh(g) = 2*sigmoid(2g) - 1, fixed up by a single
  fused DVE tensor_scalar (x*2-1).
- The input projection x @ Wi uses fp8 DoubleRow matmuls (K=256 packed over
  two of the three 128-chunks of A, plus one plain fp8 K=128 matmul).
- Batch 512 per core is processed as two interleaved halves of 256 whose
  recurrent chains are software-pipelined half a step apart (half B's
  tanh(c)/h for step t-1 are emitted inside iteration t) so the strict-FIFO
  ACT/DVE queues never serialize the two chains.
"""

import sys

sys.path.insert(0, "/opt/trn_rl_repo")

import numpy as np
import ml_dtypes

import concourse.bass as bass
import concourse.mybir as mybir
import concourse.tile as tile
from concourse import bacc
from concourse.bass_utils import run_bass_kernel_spmd

dt = mybir.dt
AF = mybir.ActivationFunctionType
ALU = mybir.AluOpType
PM = mybir.MatmulPerfMode

NCORES = 8
BFULL = 4096
B = BFULL // NCORES  # 512 batch rows per core
T = 128
F = 128
G = 512  # 4F
TC = 16  # time steps per DMA chunk
NH = B // 2  # half-batch = 256

USE_FP8 = True  # x + Wi in fp8e4m3 with DoubleRow matmuls; else bf16
DEBUG_DUMP = False  # add h/c state dram outputs (CoreSim debugging)

E4 = ml_dtypes.float8_e4m3
BF = ml_dtypes.bfloat16


def build_nc(zero_bias: bool):
    nc = bacc.Bacc("TRN2", target_bir_lowering=False, debug=False, num_devices=NCORES)

    xdt = dt.float8e4 if USE_FP8 else dt.bfloat16
    seq = nc.dram_tensor("seq", [F, T, B], xdt, kind="ExternalInput")
    seq_e = nc.dram_tensor("seq_e", [F, T, B], xdt, kind="ExternalInput")
    seq_t = nc.dram_tensor("seq_t", [F, T, B], xdt, kind="ExternalInput")
    srcT = nc.dram_tensor("srcT", [F, B], dt.bfloat16, kind="ExternalInput")
    Wi = nc.dram_tensor("Wi", [128, 3, G], xdt, kind="ExternalInput")
    Wh = nc.dram_tensor("Wh", [F, G], dt.bfloat16, kind="ExternalInput")
    bh = nc.dram_tensor("bh", [128, 4], dt.float32, kind="ExternalInput")
    W1 = nc.dram_tensor("W1", [128, 2, F], dt.bfloat16, kind="ExternalInput")
    b1 = nc.dram_tensor("b1", [F, 1], dt.float32, kind="ExternalInput")
    W2 = nc.dram_tensor("W2", [F, F], dt.bfloat16, kind="ExternalInput")
    b2 = nc.dram_tensor("b2", [F, 1], dt.float32, kind="ExternalInput")
    outT = nc.dram_tensor("outT", [F, B], dt.float32, kind="ExternalOutput")

    xdram = [seq, seq_e, seq_t]
    nchunk = T // TC

    with tile.TileContext(nc) as tc:
        with (
            tc.tile_pool(name="const", bufs=1) as constp,
            tc.tile_pool(name="xt", bufs=2) as xtp,
            tc.tile_pool(name="gates", bufs=2) as gatep,
        ):
            # ---------------- critical-path loads first ----------------
            # Wi, then the first few timesteps of x, then Wh — so the first
            # input-projection matmuls start ~12us earlier than if all the
            # small constant loads queued ahead of the x data.
            wi = constp.tile([128, 3, G], xdt)
            nc.sync.dma_start(wi[:], Wi[:])

            xtiles = {}

            def load_chunk(ch, split=1):
                xt = xtp.tile([128, 3, TC, B], xdt, tag="xt", name=f"xt_{ch}")
                sub = TC // split
                for s in range(split):
                    tsl = slice(s * sub, (s + 1) * sub)
                    for kc, dram in enumerate(xdram):
                        # prologue loads alternate between the two HWDGE
                        # rings (SP + ACT) so they drain in parallel
                        eng = nc.scalar if (split > 1 and (s * 3 + kc) % 2) else nc.sync
                        eng.dma_start(
                            xt[:, kc, tsl, :],
                            dram[:, ch * TC + s * sub : ch * TC + (s + 1) * sub, :],
                        )
                xtiles[ch] = xt

            load_chunk(0, split=8)

            wh = constp.tile([128, G], dt.bfloat16)
            nc.sync.dma_start(wh[:], Wh[:])

            # ---------------- remaining constants ----------------
            w1 = constp.tile([128, 2, F], dt.bfloat16)
            nc.sync.dma_start(w1[:], W1[:])
            w2 = constp.tile([128, F], dt.bfloat16)
            nc.sync.dma_start(w2[:], W2[:])
            b1t = constp.tile([128, 1], dt.float32)
            nc.sync.dma_start(b1t[:], b1[:])
            b2t = constp.tile([128, 1], dt.float32)
            nc.sync.dma_start(b2t[:], b2[:])
            bh4 = constp.tile([128, 4], dt.float32)
            nc.sync.dma_start(bh4[:], bh[:])
            srt = constp.tile([128, B], dt.bfloat16)
            nc.sync.dma_start(srt[:], srcT[:])

            # ---------------- persistent state ----------------
            cs = []
            hs = []
            for hf in range(2):
                c_h = constp.tile([128, NH], dt.bfloat16, name=f"c_{hf}")
                nc.gpsimd.memset(c_h[:], 0.0)
                cs.append(c_h)
                h_h = constp.tile([128, NH], dt.bfloat16, name=f"h_{hf}")
                nc.gpsimd.memset(h_h[:], 0.0)
                hs.append(h_h)

            def wi_mms(z, xt, ts, hf):
                """Input projection for one half-step into a [128,4,NH] z tile.

                Banks: quads (0,1) share bank 0, quads (2,3) share bank 1 —
                start=True only on the first matmul touching each bank; stop
                comes later from the half's own Wh matmuls.
                """
                hsl = slice(hf * NH, (hf + 1) * NH)
                for q in range(4):
                    qs = slice(q * 128, (q + 1) * 128)
                    first = q in (0, 2)
                    if USE_FP8:
                        nc.tensor.matmul(
                            z[:, q, :],
                            wi[:, 0:2, qs],
                            xt[:, 0:2, ts, hsl],
                            start=first,
                            stop=False,
                            perf_mode=PM.DoubleRow,
                        )
                        nc.tensor.matmul(
                            z[:, q, :],
                            wi[:, 2, qs],
                            xt[:, 2, ts, hsl],
                            start=False,
                            stop=False,
                        )
                    else:
                        for kc in range(3):
                            nc.tensor.matmul(
                                z[:, q, :],
                                wi[:, kc, qs],
                                xt[:, kc, ts, hsl],
                                start=(first and kc == 0),
                                stop=False,
                            )

            def wh_mms(z, hf):
                for q in range(4):
                    nc.tensor.matmul(
                        z[:, q, :],
                        wh[:, q * 128 : (q + 1) * 128],
                        hs[hf][:],
                        start=False,
                        stop=(q in (1, 3)),  # last matmul of each bank
                    )

            def sig_gates(z, hf, t):
                """sigmoid over all 4 quadrants of one half's z -> sg tile."""
                sg = gatep.tile(
                    [128, 4, NH], dt.bfloat16, tag=f"sg{hf}", name=f"sg{hf}_{t}"
                )
                if zero_bias:
                    nc.scalar.activation(sg[:], z[:], AF.Sigmoid)
                else:
                    for q in range(4):
                        nc.scalar.activation(
                            sg[:, q, :],
                            z[:, q, :],
                            AF.Sigmoid,
                            bias=bh4[:, q : q + 1],
                        )
                return sg

            def cell_update(sg, hf, t):
                """tg = 2*sig(2g)-1; c = sig(f)*c + sig(i)*tg  (DVE only)."""
                tg = gatep.tile([128, NH], dt.bfloat16, tag=f"tg{hf}", name=f"tg{hf}_{t}")
                nc.vector.tensor_scalar(tg[:], sg[:, 3, :], 2.0, -1.0, ALU.mult, ALU.add)
                m1 = gatep.tile([128, NH], dt.bfloat16, tag=f"m1{hf}", name=f"m1{hf}_{t}")
                nc.vector.tensor_mul(m1[:], sg[:, 1, :], cs[hf][:])
                m2 = gatep.tile([128, NH], dt.bfloat16, tag=f"m2{hf}", name=f"m2{hf}_{t}")
                nc.vector.tensor_mul(m2[:], sg[:, 0, :], tg[:])
                nc.vector.tensor_add(cs[hf][:], m1[:], m2[:])

            def h_update(sg, hf, t):
                """tc = tanh(c) (ACT); h = sig(o)*tc (DVE, split in halves)."""
                tc_ = gatep.tile([128, NH], dt.bfloat16, tag=f"tc{hf}", name=f"tc{hf}_{t}")
                nc.scalar.activation(tc_[:], cs[hf][:], AF.Tanh)
                nc.vector.tensor_mul(hs[hf][:], sg[:, 2, :], tc_[:])

            # ---------------- main loop ----------------
            zp_ctx = tc.tile_pool(name="zp", bufs=2, space="PSUM")
            zp = zp_ctx.__enter__()

            def z_tile(hf, t):
                return zp.tile(
                    [128, 4, NH], dt.float32, tag=f"z{hf}", name=f"z{hf}_{t}"
                )

            # HAM warm-up: keep the PE busy with throwaway matmuls on the
            # zero h/c tiles while the first x chunk streams in, so the
            # prologue input projections run at 2.4 GHz instead of 1.2.
            warm = zp.tile([128, 4, NH], dt.float32, tag="z0", name="warm")
            NWARM = 4
            for r in range(NWARM):
                for q in range(4):
                    nc.tensor.matmul(
                        warm[:, q, :],
                        hs[0][:, 0:128],
                        hs[1][:],
                        start=(r == 0 and q in (0, 2)),
                        stop=(r == NWARM - 1 and q in (1, 3)),
                    )

            zA_cur = z_tile(0, 0)
            wi_mms(zA_cur, xtiles[0], 0, 0)
            zB_cur = z_tile(1, 0)
            wi_mms(zB_cur, xtiles[0], 0, 1)

            sgB_prev = None
            for t in range(T):
                ch, ts = divmod(t, TC)
                if ts == 0 and ch + 1 < nchunk:
                    load_chunk(ch + 1)

                # half A, step t: recurrent matmul + gates + cell update
                wh_mms(zA_cur, 0)
                sgA = sig_gates(zA_cur, 0, t)
                cell_update(sgA, 0, t)

                # input projection for half A, step t+1 (fills PE idle slot)
                if t + 1 < T:
                    zA_next = z_tile(0, t + 1)
                    wi_mms(zA_next, xtiles[(t + 1) // TC], (t + 1) % TC, 0)

                # half B, step t-1: tanh(c)/h displaced into this iteration
                if sgB_prev is not None:
                    h_update(sgB_prev, 1, t - 1)

                # half B, step t: recurrent matmul
                wh_mms(zB_cur, 1)

                # half A, step t: h update
                h_update(sgA, 0, t)

                sgB = sig_gates(zB_cur, 1, t)
                cell_update(sgB, 1, t)
                sgB_prev = sgB

                # input projection for half B, step t+1
                if t + 1 < T:
                    zB_next = z_tile(1, t + 1)
                    wi_mms(zB_next, xtiles[(t + 1) // TC], (t + 1) % TC, 1)
                    zA_cur = zA_next
                    zB_cur = zB_next

            # epilogue: half B's final h
            h_update(sgB_prev, 1, T - 1)

            zp_ctx.__exit__(None, None, None)

            if DEBUG_DUMP:
                for nm, tiles in (("h", hs), ("c", cs)):
                    dbg = nc.dram_tensor(
                        f"dbg_{nm}", [F, B], dt.float32, kind="ExternalOutput"
                    )
                    sb = constp.tile([128, B], dt.float32, name=f"dbg_{nm}_sb")
                    for hf in range(2):
                        nc.vector.tensor_copy(
                            sb[:, hf * NH : (hf + 1) * NH], tiles[hf][:]
                        )
                    nc.sync.dma_start(dbg[:], sb[:])

            # ---------------- merge layer ----------------
            with tc.tile_pool(name="mp", bufs=1, space="PSUM") as mp:
                ps_hid = mp.tile([128, B], dt.float32)
                for hf in range(2):
                    nc.tensor.matmul(
                        ps_hid[:, hf * NH : (hf + 1) * NH],
                        w1[:, 0, :],
                        hs[hf][:],
                        start=(hf == 0),
                        stop=False,
                    )
                nc.tensor.matmul(ps_hid[:], w1[:, 1, :], srt[:], start=False, stop=True)
                hid_bf = constp.tile([128, B], dt.bfloat16)
                nc.scalar.activation(hid_bf[:], ps_hid[:], AF.Relu, bias=b1t[:])

                ps_out = mp.tile([128, B], dt.float32)
                nc.tensor.matmul(ps_out[:], w2[:], hid_bf[:], start=True, stop=True)
                out_sb = constp.tile([128, B], dt.float32)
                nc.scalar.activation(out_sb[:], ps_out[:], AF.Identity, bias=b2t[:])
                nc.sync.dma_start(outT[:], out_sb[:])

    nc.compile()
    return nc


_NC_CACHE: dict = {}


def _get_nc(zero_bias: bool):
    if zero_bias not in _NC_CACHE:
        _NC_CACHE[zero_bias] = build_nc(zero_bias)
    return _NC_CACHE[zero_bias]


# gate-column permutation: reference order [i, f, g, o] -> device [i, f, o, g],
# with the g columns pre-scaled by 2 (tanh(g) = 2*sig(2g) - 1 on device).
def _permute_gates(w, scale_g=True):
    i, f, g, o = np.split(np.asarray(w, dtype=np.float32), 4, axis=-1)
    return np.concatenate([i, f, o, (2.0 * g) if scale_g else g], axis=-1)


def make_in_maps(**inputs):
    """Host-side packing: per-core [F,T,B] transposed inputs + cast weights."""
    xdt = E4 if USE_FP8 else BF
    f32 = lambda x: np.asarray(x, dtype=np.float32)

    Wi = _permute_gates(f32(inputs["Wi"]))  # [384, 512]
    Wi_dev = np.ascontiguousarray(
        Wi.reshape(3, 128, G).transpose(1, 0, 2)
    ).astype(xdt)
    Wh_dev = np.ascontiguousarray(_permute_gates(f32(inputs["Wh"]))).astype(BF)
    bh_dev = np.ascontiguousarray(
        _permute_gates(f32(inputs["bh"]).reshape(1, 4 * F)).reshape(4, F).T
    ).astype(np.float32)  # [128, 4]
    W1_dev = np.ascontiguousarray(
        f32(inputs["W1"]).reshape(2, 128, F).transpose(1, 0, 2)
    ).astype(BF)
    W2_dev = np.ascontiguousarray(f32(inputs["W2"])).astype(BF)
    b1_dev = f32(inputs["b1"]).reshape(F, 1)
    b2_dev = f32(inputs["b2"]).reshape(F, 1)

    seq8 = np.asarray(inputs["seq"], dtype=np.float32).astype(xdt)
    seqe8 = np.asarray(inputs["seq_e"], dtype=np.float32).astype(xdt)
    seqt8 = np.asarray(inputs["seq_t"], dtype=np.float32).astype(xdt)
    src = f32(inputs["src"])

    shared = {
        "Wi": Wi_dev, "Wh": Wh_dev, "bh": bh_dev,
        "W1": W1_dev, "b1": b1_dev, "W2": W2_dev, "b2": b2_dev,
    }
    in_maps = []
    for c in range(NCORES):
        sl = slice(c * B, (c + 1) * B)
        m = dict(shared)
        m["seq"] = np.ascontiguousarray(seq8[sl].transpose(2, 1, 0))
        m["seq_e"] = np.ascontiguousarray(seqe8[sl].transpose(2, 1, 0))
        m["seq_t"] = np.ascontiguousarray(seqt8[sl].transpose(2, 1, 0))
        m["srcT"] = np.ascontiguousarray(src[sl].T).astype(BF)
        in_maps.append(m)
    return in_maps


def kernel(**inputs) -> np.ndarray:
    zero_bias = not np.any(np.asarray(inputs["bh"]))
    nc = _get_nc(zero_bias)
    in_maps = make_in_maps(**inputs)
    res = run_bass_kernel_spmd(nc, in_maps, core_ids=list(range(NCORES)))
    out = np.empty((BFULL, F), np.float32)
    for c in range(NCORES):
        out[c * B : (c + 1) * B] = res.results[c]["outT"].T
    return out


# revision 21
# speedup vs baseline: 1.0182x; 1.0170x over previous
"""LSTM-pool kernel for Trainium2, 8-core data-parallel SPMD.

Math (per batch row b):
  x_t = [seq[b,t], seq_e[b,t], seq_t[b,t]]              (A = 384)
  z_t = x_t @ Wi + h_{t-1} @ Wh + bh                    (4F = 512, gates i,f,g,o)
  c_t = sig(f)*c_{t-1} + sig(i)*tanh(g);  h_t = sig(o)*tanh(c_t)
  out = relu([h_T, src] @ W1 + b1) @ W2 + b2

Design notes:
- Everything on-device runs transposed: feature/gate dim on partitions, batch
  on the free dim.  The host pre-transposes each core's inputs to [F, T, B]
  and pre-casts them (x and Wi to fp8 e4m3, the rest to bf16), so the kernel
  issues plain HWDGE loads with zero on-chip transposes or casts.
- Gate columns are host-permuted to [i, f, o, g] and the g columns of Wi/Wh
  (and bh) are pre-scaled by 2 so that one sigmoid instruction covers all
  four gate quadrants: tanh(g) = 2*sigmoid(2g) - 1, fixed up by a single
  fused DVE tensor_scalar (x*2-1).
- The input projection x @ Wi uses fp8 DoubleRow matmuls (K=256 packed over
  two of the three 128-chunks of A, plus one plain fp8 K=128 matmul).
- Batch 512 per core is processed as two interleaved halves of 256 whose
  recurrent chains are software-pipelined half a step apart (half B's
  tanh(c)/h for step t-1 are emitted inside iteration t) so the strict-FIFO
  ACT/DVE queues never serialize the two chains.
"""

import sys

sys.path.insert(0, "/opt/trn_rl_repo")

import numpy as np
import ml_dtypes

import concourse.bass as bass
import concourse.mybir as mybir
import concourse.tile as tile
from concourse import bacc
from concourse.bass_utils import run_bass_kernel_spmd

dt = mybir.dt
AF = mybir.ActivationFunctionType
ALU = mybir.AluOpType
PM = mybir.MatmulPerfMode

NCORES = 8
BFULL = 4096
B = BFULL // NCORES  # 512 batch rows per core
T = 128
F = 128
G = 512  # 4F
TC = 16  # time steps per DMA chunk
NH = B // 2  # half-batch = 256

USE_FP8 = True  # x + Wi in fp8e4m3 with DoubleRow matmuls; else bf16
DEBUG_DUMP = False  # add h/c state dram outputs (CoreSim debugging)

E4 = ml_dtypes.float8_e4m3
BF = ml_dtypes.bfloat16


def build_nc(zero_bias: bool):
    nc = bacc.Bacc("TRN2", target_bir_lowering=False, debug=False, num_devices=NCORES)

    xdt = dt.float8e4 if USE_FP8 else dt.bfloat16
    seq = nc.dram_tensor("seq", [F, T, B], xdt, kind="ExternalInput")
    seq_e = nc.dram_tensor("seq_e", [F, T, B], xdt, kind="ExternalInput")
    seq_t = nc.dram_tensor("seq_t", [F, T, B], xdt, kind="ExternalInput")
    srcT = nc.dram_tensor("srcT", [F, B], dt.bfloat16, kind="ExternalInput")
    Wi = nc.dram_tensor("Wi", [128, 3, G], xdt, kind="ExternalInput")
    Wh = nc.dram_tensor("Wh", [F, G], dt.bfloat16, kind="ExternalInput")
    bh = nc.dram_tensor("bh", [128, 4], dt.float32, kind="ExternalInput")
    W1 = nc.dram_tensor("W1", [128, 2, F], dt.bfloat16, kind="ExternalInput")
    b1 = nc.dram_tensor("b1", [F, 1], dt.float32, kind="ExternalInput")
    W2 = nc.dram_tensor("W2", [F, F], dt.bfloat16, kind="ExternalInput")
    b2 = nc.dram_tensor("b2", [F, 1], dt.float32, kind="ExternalInput")
    outT = nc.dram_tensor("outT", [F, B], dt.float32, kind="ExternalOutput")

    xdram = [seq, seq_e, seq_t]
    nchunk = T // TC

    with tile.TileContext(nc) as tc:
        with (
            tc.tile_pool(name="const", bufs=1) as constp,
            tc.tile_pool(name="xt", bufs=2) as xtp,
            tc.tile_pool(name="gates", bufs=2) as gatep,
        ):
            # ---------------- critical-path loads first ----------------
            # Wi, then the first few timesteps of x, then Wh — so the first
            # input-projection matmuls start ~12us earlier than if all the
            # small constant loads queued ahead of the x data.
            wi = constp.tile([128, 3, G], xdt)
            nc.sync.dma_start(wi[:], Wi[:])

            xtiles = {}

            def load_slices(xt, ch, s0, s1, sub):
                for s in range(s0, s1):
                    tsl = slice(s * sub, (s + 1) * sub)
                    for kc, dram in enumerate(xdram):
                        nc.sync.dma_start(
                            xt[:, kc, tsl, :],
                            dram[:, ch * TC + s * sub : ch * TC + (s + 1) * sub, :],
                        )

            def load_chunk(ch):
                xt = xtp.tile([128, 3, TC, B], xdt, tag="xt", name=f"xt_{ch}")
                load_slices(xt, ch, 0, 1, TC)
                xtiles[ch] = xt

            # chunk 0 split into 2-step slices; Wh lands right after the
            # first two slices so the step-0 recurrent matmul isn't stuck
            # behind the whole chunk.
            xt0 = xtp.tile([128, 3, TC, B], xdt, tag="xt", name="xt_0")
            load_slices(xt0, 0, 0, 2, 2)
            wh = constp.tile([128, G], dt.bfloat16)
            nc.sync.dma_start(wh[:], Wh[:])
            load_slices(xt0, 0, 2, 8, 2)
            xtiles[0] = xt0

            # ---------------- remaining constants ----------------
            w1 = constp.tile([128, 2, F], dt.bfloat16)
            nc.sync.dma_start(w1[:], W1[:])
            w2 = constp.tile([128, F], dt.bfloat16)
            nc.sync.dma_start(w2[:], W2[:])
            b1t = constp.tile([128, 1], dt.float32)
            nc.sync.dma_start(b1t[:], b1[:])
            b2t = constp.tile([128, 1], dt.float32)
            nc.sync.dma_start(b2t[:], b2[:])
            bh4 = constp.tile([128, 4], dt.float32)
            nc.sync.dma_start(bh4[:], bh[:])
            srt = constp.tile([128, B], dt.bfloat16)
            nc.sync.dma_start(srt[:], srcT[:])

            # ---------------- persistent state ----------------
            cs = []
            hs = []
            for hf in range(2):
                c_h = constp.tile([128, NH], dt.bfloat16, name=f"c_{hf}")
                nc.gpsimd.memset(c_h[:], 0.0)
                cs.append(c_h)
                h_h = constp.tile([128, NH], dt.bfloat16, name=f"h_{hf}")
                nc.gpsimd.memset(h_h[:], 0.0)
                hs.append(h_h)

            def wi_mms(z, xt, ts, hf):
                """Input projection for one half-step into a [128,4,NH] z tile.

                Banks: quads (0,1) share bank 0, quads (2,3) share bank 1 —
                start=True only on the first matmul touching each bank; stop
                comes later from the half's own Wh matmuls.
                """
                hsl = slice(hf * NH, (hf + 1) * NH)
                for q in range(4):
                    qs = slice(q * 128, (q + 1) * 128)
                    first = q in (0, 2)
                    if USE_FP8:
                        nc.tensor.matmul(
                            z[:, q, :],
                            wi[:, 0:2, qs],
                            xt[:, 0:2, ts, hsl],
                            start=first,
                            stop=False,
                            perf_mode=PM.DoubleRow,
                        )
                        nc.tensor.matmul(
                            z[:, q, :],
                            wi[:, 2, qs],
                            xt[:, 2, ts, hsl],
                            start=False,
                            stop=False,
                        )
                    else:
                        for kc in range(3):
                            nc.tensor.matmul(
                                z[:, q, :],
                                wi[:, kc, qs],
                                xt[:, kc, ts, hsl],
                                start=(first and kc == 0),
                                stop=False,
                            )

            def wh_mms(z, hf):
                for q in range(4):
                    nc.tensor.matmul(
                        z[:, q, :],
                        wh[:, q * 128 : (q + 1) * 128],
                        hs[hf][:],
                        start=False,
                        stop=(q in (1, 3)),  # last matmul of each bank
                    )

            def sig_gates(z, hf, t):
                """sigmoid over all 4 quadrants of one half's z -> sg tile."""
                sg = gatep.tile(
                    [128, 4, NH], dt.bfloat16, tag=f"sg{hf}", name=f"sg{hf}_{t}"
                )
                if zero_bias:
                    nc.scalar.activation(sg[:], z[:], AF.Sigmoid)
                else:
                    for q in range(4):
                        nc.scalar.activation(
                            sg[:, q, :],
                            z[:, q, :],
                            AF.Sigmoid,
                            bias=bh4[:, q : q + 1],
                        )
                return sg

            def cell_update(sg, hf, t):
                """tg = 2*sig(2g)-1; c = sig(f)*c + sig(i)*tg  (DVE only)."""
                tg = gatep.tile([128, NH], dt.bfloat16, tag=f"tg{hf}", name=f"tg{hf}_{t}")
                nc.vector.tensor_scalar(tg[:], sg[:, 3, :], 2.0, -1.0, ALU.mult, ALU.add)
                m1 = gatep.tile([128, NH], dt.bfloat16, tag=f"m1{hf}", name=f"m1{hf}_{t}")
                nc.vector.tensor_mul(m1[:], sg[:, 1, :], cs[hf][:])
                m2 = gatep.tile([128, NH], dt.bfloat16, tag=f"m2{hf}", name=f"m2{hf}_{t}")
                nc.vector.tensor_mul(m2[:], sg[:, 0, :], tg[:])
                nc.vector.tensor_add(cs[hf][:], m1[:], m2[:])

            def h_update(sg, hf, t):
                """tc = tanh(c) (ACT); h = sig(o)*tc (DVE, split in halves)."""
                tc_ = gatep.tile([128, NH], dt.bfloat16, tag=f"tc{hf}", name=f"tc{hf}_{t}")
                nc.scalar.activation(tc_[:], cs[hf][:], AF.Tanh)
                nc.vector.tensor_mul(hs[hf][:], sg[:, 2, :], tc_[:])

            # ---------------- main loop ----------------
            zp_ctx = tc.tile_pool(name="zp", bufs=2, space="PSUM")
            zp = zp_ctx.__enter__()

            def z_tile(hf, t):
                return zp.tile(
                    [128, 4, NH], dt.float32, tag=f"z{hf}", name=f"z{hf}_{t}"
                )

            # HAM warm-up: keep the PE busy with throwaway matmuls on the
            # zero h/c tiles while the first x chunk streams in, so the
            # prologue input projections run at 2.4 GHz instead of 1.2.
            warm = zp.tile([128, 4, NH], dt.float32, tag="z0", name="warm")
            NWARM = 4
            for r in range(NWARM):
                for q in range(4):
                    nc.tensor.matmul(
                        warm[:, q, :],
                        hs[0][:, 0:128],
                        hs[1][:],
                        start=(r == 0 and q in (0, 2)),
                        stop=(r == NWARM - 1 and q in (1, 3)),
                    )

            zA_cur = z_tile(0, 0)
            wi_mms(zA_cur, xtiles[0], 0, 0)
            zB_cur = z_tile(1, 0)
            wi_mms(zB_cur, xtiles[0], 0, 1)

            sgB_prev = None
            for t in range(T):
                ch, ts = divmod(t, TC)
                if ts == 0 and ch + 1 < nchunk:
                    load_chunk(ch + 1)

                # half A, step t: recurrent matmul + gates + cell update
                wh_mms(zA_cur, 0)
                sgA = sig_gates(zA_cur, 0, t)
                cell_update(sgA, 0, t)

                # input projection for half A, step t+1 (fills PE idle slot)
                if t + 1 < T:
                    zA_next = z_tile(0, t + 1)
                    wi_mms(zA_next, xtiles[(t + 1) // TC], (t + 1) % TC, 0)

                # half B, step t-1: tanh(c)/h displaced into this iteration
                if sgB_prev is not None:
                    h_update(sgB_prev, 1, t - 1)

                # half B, step t: recurrent matmul
                wh_mms(zB_cur, 1)

                # half A, step t: h update
                h_update(sgA, 0, t)

                sgB = sig_gates(zB_cur, 1, t)
                cell_update(sgB, 1, t)
                sgB_prev = sgB

                # input projection for half B, step t+1
                if t + 1 < T:
                    zB_next = z_tile(1, t + 1)
                    wi_mms(zB_next, xtiles[(t + 1) // TC], (t + 1) % TC, 1)
                    zA_cur = zA_next
                    zB_cur = zB_next

            # epilogue: half B's final h
            h_update(sgB_prev, 1, T - 1)

            zp_ctx.__exit__(None, None, None)

            if DEBUG_DUMP:
                for nm, tiles in (("h", hs), ("c", cs)):
                    dbg = nc.dram_tensor(
                        f"dbg_{nm}", [F, B], dt.float32, kind="ExternalOutput"
                    )
                    sb = constp.tile([128, B], dt.float32, name=f"dbg_{nm}_sb")
                    for hf in range(2):
                        nc.vector.tensor_copy(
                            sb[:, hf * NH : (hf + 1) * NH], tiles[hf][:]
                        )
                    nc.sync.dma_start(dbg[:], sb[:])

            # ---------------- merge layer ----------------
            with tc.tile_pool(name="mp", bufs=1, space="PSUM") as mp:
                ps_hid = mp.tile([128, B], dt.float32)
                for hf in range(2):
                    nc.tensor.matmul(
                        ps_hid[:, hf * NH : (hf + 1) * NH],
                        w1[:, 0, :],
                        hs[hf][:],
                        start=(hf == 0),
                        stop=False,
                    )
                nc.tensor.matmul(ps_hid[:], w1[:, 1, :], srt[:], start=False, stop=True)
                hid_bf = constp.tile([128, B], dt.bfloat16)
                nc.scalar.activation(hid_bf[:], ps_hid[:], AF.Relu, bias=b1t[:])

                ps_out = mp.tile([128, B], dt.float32)
                nc.tensor.matmul(ps_out[:], w2[:], hid_bf[:], start=True, stop=True)
                out_sb = constp.tile([128, B], dt.float32)
                nc.scalar.activation(out_sb[:], ps_out[:], AF.Identity, bias=b2t[:])
                nc.sync.dma_start(outT[:], out_sb[:])

    nc.compile()
    return nc


_NC_CACHE: dict = {}


def _get_nc(zero_bias: bool):
    if zero_bias not in _NC_CACHE:
        _NC_CACHE[zero_bias] = build_nc(zero_bias)
    return _NC_CACHE[zero_bias]


# gate-column permutation: reference order [i, f, g, o] -> device [i, f, o, g],
# with the g columns pre-scaled by 2 (tanh(g) = 2*sig(2g) - 1 on device).
def _permute_gates(w, scale_g=True):
    i, f, g, o = np.split(np.asarray(w, dtype=np.float32), 4, axis=-1)
    return np.concatenate([i, f, o, (2.0 * g) if scale_g else g], axis=-1)


def make_in_maps(**inputs):
    """Host-side packing: per-core [F,T,B] transposed inputs + cast weights."""
    xdt = E4 if USE_FP8 else BF
    f32 = lambda x: np.asarray(x, dtype=np.float32)

    Wi = _permute_gates(f32(inputs["Wi"]))  # [384, 512]
    Wi_dev = np.ascontiguousarray(
        Wi.reshape(3, 128, G).transpose(1, 0, 2)
    ).astype(xdt)
    Wh_dev = np.ascontiguousarray(_permute_gates(f32(inputs["Wh"]))).astype(BF)
    bh_dev = np.ascontiguousarray(
        _permute_gates(f32(inputs["bh"]).reshape(1, 4 * F)).reshape(4, F).T
    ).astype(np.float32)  # [128, 4]
    W1_dev = np.ascontiguousarray(
        f32(inputs["W1"]).reshape(2, 128, F).transpose(1, 0, 2)
    ).astype(BF)
    W2_dev = np.ascontiguousarray(f32(inputs["W2"])).astype(BF)
    b1_dev = f32(inputs["b1"]).reshape(F, 1)
    b2_dev = f32(inputs["b2"]).reshape(F, 1)

    seq8 = np.asarray(inputs["seq"], dtype=np.float32).astype(xdt)
    seqe8 = np.asarray(inputs["seq_e"], dtype=np.float32).astype(xdt)
    seqt8 = np.asarray(inputs["seq_t"], dtype=np.float32).astype(xdt)
    src = f32(inputs["src"])

    shared = {
        "Wi": Wi_dev, "Wh": Wh_dev, "bh": bh_dev,
        "W1": W1_dev, "b1": b1_dev, "W2": W2_dev, "b2": b2_dev,
    }
    in_maps = []
    for c in range(NCORES):
        sl = slice(c * B, (c + 1) * B)
        m = dict(shared)
        m["seq"] = np.ascontiguousarray(seq8[sl].transpose(2, 1, 0))
        m["seq_e"] = np.ascontiguousarray(seqe8[sl].transpose(2, 1, 0))
        m["seq_t"] = np.ascontiguousarray(seqt8[sl].transpose(2, 1, 0))
        m["srcT"] = np.ascontiguousarray(src[sl].T).astype(BF)
        in_maps.append(m)
    return in_maps


def kernel(**inputs) -> np.ndarray:
    zero_bias = not np.any(np.asarray(inputs["bh"]))
    nc = _get_nc(zero_bias)
    in_maps = make_in_maps(**inputs)
    res = run_bass_kernel_spmd(nc, in_maps, core_ids=list(range(NCORES)))
    out = np.empty((BFULL, F), np.float32)
    for c in range(NCORES):
        out[c * B : (c + 1) * B] = res.results[c]["outT"].T
    return out
